# revision 12
# baseline (speedup 1.0000x reference)
"""Trainium2 Bass kernel for nn_L2LossDif (pairwise L2 contrastive loss).

Math (see the algebraic reduction in the problem's reference):
    sq_m  = sum(feats_m ** 2)           (scalar, per matrix)
    mu_m  = feats_m.sum(axis=0)         ([D], per matrix)
then a handful of scalar ops combine sq_n, sq_a, mu_n, mu_a into the loss.

Strategy: data-parallel row shard across 8 cores (1024 rows of each matrix
per core). Each core streams its 16 MiB of rows once from HBM (2 MiB HWDGE
chunks). Per-chunk work is split so every engine runs faster than the DMA:
  - sum of squares     : ScalarE Square activation with accum_out
  - column sums 0:1024 : TensorE ones-matmul (float32r, 1 cyc/row) -> PSUM
  - column sums 1024:  : VectorE adds into a [128, 1024] accumulator
The DMA stream is the roofline. Partition/core reductions and the scalar
combine run on the host in float64.
"""

import numpy as np

import concourse.bacc as bacc
import concourse.mybir as mybir
import concourse.tile as tile
from concourse.bass_utils import run_bass_kernel_spmd

N_CORES = 8
N_ROWS_FULL = 8192
D = 2048
P = 128
ROWS = N_ROWS_FULL // N_CORES  # rows per core per matrix
K_TILES = 2  # 128-row tiles per DMA chunk (2 -> 2 MiB chunks)
CHUNK_ROWS = P * K_TILES
NCHUNK = ROWS // CHUNK_ROWS  # chunks per matrix
MM_N = 512  # moving free dim per matmul
D_PE = 1024  # columns summed on TensorE; the rest go to VectorE

_NC_CACHE = {}


def build_module():
    nc = bacc.Bacc("TRN2", target_bir_lowering=False, debug=False)
    f32 = mybir.dt.float32
    f32r = mybir.dt.float32r
    srcs = [
        nc.dram_tensor("nfeats", [ROWS, D], f32, kind="ExternalInput"),
        nc.dram_tensor("afeats", [ROWS, D], f32, kind="ExternalInput"),
    ]
    out_mulo = nc.dram_tensor("mulo", [2, D_PE], f32, kind="ExternalOutput")
    out_acchi = nc.dram_tensor("acchi", [2, P, D - D_PE], f32, kind="ExternalOutput")
    out_rsq = nc.dram_tensor("rsq", [P, 2 * NCHUNK], f32, kind="ExternalOutput")

    with tile.TileContext(nc) as tc:
        with (
            tc.tile_pool(name="chunks", bufs=8) as chunk_pool,
            tc.tile_pool(name="sq", bufs=2) as sq_pool,
            tc.tile_pool(name="psum", bufs=1, space="PSUM") as psum_pool,
            tc.tile_pool(name="small", bufs=1) as small_pool,
        ):
            rsq_all = small_pool.tile([P, 2 * NCHUNK], f32)
            ones = small_pool.tile([P, 1], f32)
            nc.gpsimd.memset(ones, 1.0)
            ones_r = ones.bitcast(f32r)

            for m, src in enumerate(srcs):
                psum_mu = psum_pool.tile([1, D_PE], f32, tag=f"psum{m}")
                acc_hi = small_pool.tile([P, D - D_PE], f32, tag=f"acchi{m}")
                nc.gpsimd.memset(acc_hi, 0.0)
                for c in range(NCHUNK):
                    chunk = chunk_pool.tile([P, K_TILES * D], f32r)
                    nc.sync.dma_start(
                        out=chunk,
                        in_=src[c * CHUNK_ROWS : (c + 1) * CHUNK_ROWS, :]
                        .rearrange("(p k) d -> p (k d)", p=P)
                        .bitcast(f32r),
                    )
                    sq = sq_pool.tile([P, K_TILES * D], mybir.dt.bfloat16)
                    nc.scalar.activation(
                        out=sq,
                        in_=chunk.bitcast(f32),
                        func=mybir.ActivationFunctionType.Square,
                        accum_out=rsq_all[:, m * NCHUNK + c : m * NCHUNK + c + 1],
                    )
                    for k in range(K_TILES):
                        for j in range(D_PE // MM_N):
                            nc.tensor.matmul(
                                psum_mu[0:1, j * MM_N : (j + 1) * MM_N],
                                lhsT=ones_r,
                                rhs=chunk[:, k * D + j * MM_N : k * D + (j + 1) * MM_N],
                                start=(c == 0 and k == 0),
                                stop=(c == NCHUNK - 1 and k == K_TILES - 1),
                            )
                        nc.vector.tensor_add(
                            acc_hi,
                            acc_hi,
                            chunk[:, k * D + D_PE : (k + 1) * D].bitcast(f32),
                        )
                mu_sb = small_pool.tile([1, D_PE], f32, tag=f"mu{m}")
                nc.vector.tensor_copy(mu_sb, psum_mu)
                # Output DMAs go on the (idle) GpSimd SWDGE queue, emitted at
                # the end: the SP sequencer runs in order, so an output DMA
                # waiting mid-stream would stall the remaining input loads.
                nc.gpsimd.dma_start(out=out_mulo[m : m + 1, :], in_=mu_sb)
                nc.gpsimd.dma_start(out=out_acchi[m], in_=acc_hi)
            nc.gpsimd.dma_start(out=out_rsq[:, :], in_=rsq_all)
    nc.compile()
    return nc


def get_module():
    if "nc" not in _NC_CACHE:
        _NC_CACHE["nc"] = build_module()
    return _NC_CACHE["nc"]


def kernel(nfeats, afeats):
    nfeats = np.asarray(nfeats, dtype=np.float32)
    afeats = np.asarray(afeats, dtype=np.float32)
    assert nfeats.shape == (N_ROWS_FULL, D) and afeats.shape == (N_ROWS_FULL, D)

    nc = get_module()
    in_maps = [
        {
            "nfeats": np.ascontiguousarray(nfeats[c * ROWS : (c + 1) * ROWS]),
            "afeats": np.ascontiguousarray(afeats[c * ROWS : (c + 1) * ROWS]),
        }
        for c in range(N_CORES)
    ]
    results = run_bass_kernel_spmd(nc, in_maps, core_ids=list(range(N_CORES))).results

    mu = np.zeros((2, D), dtype=np.float64)
    sq = np.zeros(2, dtype=np.float64)
    for r in results:
        mu[:, :D_PE] += np.asarray(r["mulo"], dtype=np.float64)
        mu[:, D_PE:] += np.asarray(r["acchi"], dtype=np.float64).sum(axis=1)
        rsq = np.asarray(r["rsq"], dtype=np.float64)
        sq[0] += rsq[:, :NCHUNK].sum()
        sq[1] += rsq[:, NCHUNK:].sum()

    return combine(mu[0], mu[1], sq[0], sq[1])


def combine(mu_n, mu_a, sq_n, sq_a):
    nnum = anum = float(N_ROWS_FULL)
    nsum = nnum * sq_n - float(mu_n @ mu_n)
    asum = anum * sq_a - float(mu_a @ mu_a)
    cross_sum = anum * sq_n + nnum * sq_a - 2.0 * float(mu_n @ mu_a)

    ncount = nnum * (nnum - 1) / 2
    acount = anum * (anum - 1) / 2
    count = nnum * anum

    loss_dif = cross_sum / count
    within = (asum + nsum) / (acount + ncount)
    loss = -np.log(loss_dif / (loss_dif + within))
    return np.asarray(loss, dtype=np.float32)


# revision 13
# speedup vs baseline: 1.0328x; 1.0328x over previous
"""Trainium2 Bass kernel for nn_L2LossDif (pairwise L2 contrastive loss).

Math (see the algebraic reduction in the problem's reference):
    sq_m  = sum(feats_m ** 2)           (scalar, per matrix)
    mu_m  = feats_m.sum(axis=0)         ([D], per matrix)
then a handful of scalar ops combine sq_n, sq_a, mu_n, mu_a into the loss.

Strategy: data-parallel row shard across 8 cores (1024 rows of each matrix
per core). Each core streams its 16 MiB of rows once from HBM (2 MiB HWDGE
chunks). Per-chunk work is split so every engine runs faster than the DMA:
  - sum of squares     : ScalarE Square activation with accum_out
  - column sums 0:1024 : TensorE ones-matmul (float32r, 1 cyc/row) -> PSUM
  - column sums 1024:  : VectorE adds into a [128, 1024] accumulator
The DMA stream is the roofline. Partition/core reductions and the scalar
combine run on the host in float64.
"""

import numpy as np

import concourse.bacc as bacc
import concourse.mybir as mybir
import concourse.tile as tile
from concourse.bass_utils import run_bass_kernel_spmd

N_CORES = 8
N_ROWS_FULL = 8192
D = 2048
P = 128
ROWS = N_ROWS_FULL // N_CORES  # rows per core per matrix
K_TILES = 1  # 128-row tiles per DMA chunk (1 -> 1 MiB chunks)
CHUNK_ROWS = P * K_TILES
NCHUNK = ROWS // CHUNK_ROWS  # chunks per matrix
MM_N = 512  # moving free dim per matmul
D_PE = 1024  # columns summed on TensorE; the rest go to VectorE

_NC_CACHE = {}


def build_module():
    nc = bacc.Bacc("TRN2", target_bir_lowering=False, debug=False)
    f32 = mybir.dt.float32
    f32r = mybir.dt.float32r
    srcs = [
        nc.dram_tensor("nfeats", [ROWS, D], f32, kind="ExternalInput"),
        nc.dram_tensor("afeats", [ROWS, D], f32, kind="ExternalInput"),
    ]
    out_mulo = nc.dram_tensor("mulo", [2, D_PE], f32, kind="ExternalOutput")
    out_acchi = nc.dram_tensor("acchi", [2, P, D - D_PE], f32, kind="ExternalOutput")
    out_rsq = nc.dram_tensor("rsq", [P, 2 * NCHUNK], f32, kind="ExternalOutput")

    with tile.TileContext(nc) as tc:
        with (
            tc.tile_pool(name="chunks", bufs=8) as chunk_pool,
            tc.tile_pool(name="sq", bufs=2) as sq_pool,
            tc.tile_pool(name="psum", bufs=1, space="PSUM") as psum_pool,
            tc.tile_pool(name="small", bufs=1) as small_pool,
        ):
            rsq_all = small_pool.tile([P, 2 * NCHUNK], f32)
            ones = small_pool.tile([P, 1], f32)
            nc.gpsimd.memset(ones, 1.0)
            ones_r = ones.bitcast(f32r)

            for m, src in enumerate(srcs):
                psum_mu = psum_pool.tile([1, D_PE], f32, tag=f"psum{m}")
                acc_hi = small_pool.tile([P, D - D_PE], f32, tag=f"acchi{m}")
                nc.gpsimd.memset(acc_hi, 0.0)
                for c in range(NCHUNK):
                    chunk = chunk_pool.tile([P, K_TILES * D], f32r)
                    nc.sync.dma_start(
                        out=chunk,
                        in_=src[c * CHUNK_ROWS : (c + 1) * CHUNK_ROWS, :]
                        .rearrange("(p k) d -> p (k d)", p=P)
                        .bitcast(f32r),
                    )
                    sq = sq_pool.tile([P, K_TILES * D], mybir.dt.bfloat16)
                    nc.scalar.activation(
                        out=sq,
                        in_=chunk.bitcast(f32),
                        func=mybir.ActivationFunctionType.Square,
                        accum_out=rsq_all[:, m * NCHUNK + c : m * NCHUNK + c + 1],
                    )
                    for k in range(K_TILES):
                        for j in range(D_PE // MM_N):
                            nc.tensor.matmul(
                                psum_mu[0:1, j * MM_N : (j + 1) * MM_N],
                                lhsT=ones_r,
                                rhs=chunk[:, k * D + j * MM_N : k * D + (j + 1) * MM_N],
                                start=(c == 0 and k == 0),
                                stop=(c == NCHUNK - 1 and k == K_TILES - 1),
                            )
                        nc.vector.tensor_add(
                            acc_hi,
                            acc_hi,
                            chunk[:, k * D + D_PE : (k + 1) * D].bitcast(f32),
                        )
                mu_sb = small_pool.tile([1, D_PE], f32, tag=f"mu{m}")
                nc.vector.tensor_copy(mu_sb, psum_mu)
                # Output DMAs go on the (idle) GpSimd SWDGE queue, emitted at
                # the end: the SP sequencer runs in order, so an output DMA
                # waiting mid-stream would stall the remaining input loads.
                nc.gpsimd.dma_start(out=out_mulo[m : m + 1, :], in_=mu_sb)
                nc.gpsimd.dma_start(out=out_acchi[m], in_=acc_hi)
            nc.gpsimd.dma_start(out=out_rsq[:, :], in_=rsq_all)
    nc.compile()
    return nc


def get_module():
    if "nc" not in _NC_CACHE:
        _NC_CACHE["nc"] = build_module()
    return _NC_CACHE["nc"]


def kernel(nfeats, afeats):
    nfeats = np.asarray(nfeats, dtype=np.float32)
    afeats = np.asarray(afeats, dtype=np.float32)
    assert nfeats.shape == (N_ROWS_FULL, D) and afeats.shape == (N_ROWS_FULL, D)

    nc = get_module()
    in_maps = [
        {
            "nfeats": np.ascontiguousarray(nfeats[c * ROWS : (c + 1) * ROWS]),
            "afeats": np.ascontiguousarray(afeats[c * ROWS : (c + 1) * ROWS]),
        }
        for c in range(N_CORES)
    ]
    results = run_bass_kernel_spmd(nc, in_maps, core_ids=list(range(N_CORES))).results

    mu = np.zeros((2, D), dtype=np.float64)
    sq = np.zeros(2, dtype=np.float64)
    for r in results:
        mu[:, :D_PE] += np.asarray(r["mulo"], dtype=np.float64)
        mu[:, D_PE:] += np.asarray(r["acchi"], dtype=np.float64).sum(axis=1)
        rsq = np.asarray(r["rsq"], dtype=np.float64)
        sq[0] += rsq[:, :NCHUNK].sum()
        sq[1] += rsq[:, NCHUNK:].sum()

    return combine(mu[0], mu[1], sq[0], sq[1])


def combine(mu_n, mu_a, sq_n, sq_a):
    nnum = anum = float(N_ROWS_FULL)
    nsum = nnum * sq_n - float(mu_n @ mu_n)
    asum = anum * sq_a - float(mu_a @ mu_a)
    cross_sum = anum * sq_n + nnum * sq_a - 2.0 * float(mu_n @ mu_a)

    ncount = nnum * (nnum - 1) / 2
    acount = anum * (anum - 1) / 2
    count = nnum * anum

    loss_dif = cross_sum / count
    within = (asum + nsum) / (acount + ncount)
    loss = -np.log(loss_dif / (loss_dif + within))
    return np.asarray(loss, dtype=np.float32)


# revision 14
# speedup vs baseline: 1.0389x; 1.0059x over previous
"""Trainium2 Bass kernel for nn_L2LossDif (pairwise L2 contrastive loss).

Math (see the algebraic reduction in the problem's reference):
    sq_m  = sum(feats_m ** 2)           (scalar, per matrix)
    mu_m  = feats_m.sum(axis=0)         ([D], per matrix)
then a handful of scalar ops combine sq_n, sq_a, mu_n, mu_a into the loss.

Strategy: data-parallel row shard across 8 cores (1024 rows of each matrix
per core). Each core streams its 16 MiB of rows once from HBM (2 MiB HWDGE
chunks). Per-chunk work is split so every engine runs faster than the DMA:
  - sum of squares     : ScalarE Square activation with accum_out
  - column sums 0:1024 : TensorE ones-matmul (float32r, 1 cyc/row) -> PSUM
  - column sums 1024:  : VectorE adds into a [128, 1024] accumulator
The DMA stream is the roofline. Partition/core reductions and the scalar
combine run on the host in float64.
"""

import numpy as np

import concourse.bacc as bacc
import concourse.mybir as mybir
import concourse.tile as tile
from concourse.bass_utils import run_bass_kernel_spmd

N_CORES = 8
N_ROWS_FULL = 8192
D = 2048
P = 128
ROWS = N_ROWS_FULL // N_CORES  # rows per core per matrix
K_TILES = 1  # 128-row tiles per DMA chunk (1 -> 1 MiB chunks)
CHUNK_ROWS = P * K_TILES
NCHUNK = ROWS // CHUNK_ROWS  # chunks per matrix
MM_N = 512  # moving free dim per matmul
D_PE = 1024  # columns summed on TensorE; the rest go to VectorE

_NC_CACHE = {}


def build_module():
    nc = bacc.Bacc("TRN2", target_bir_lowering=False, debug=False)
    f32 = mybir.dt.float32
    f32r = mybir.dt.float32r
    srcs = [
        nc.dram_tensor("nfeats", [ROWS, D], f32, kind="ExternalInput"),
        nc.dram_tensor("afeats", [ROWS, D], f32, kind="ExternalInput"),
    ]
    out_mulo = nc.dram_tensor("mulo", [2, D_PE], f32, kind="ExternalOutput")
    out_acchi = nc.dram_tensor("acchi", [2, P, D - D_PE], f32, kind="ExternalOutput")
    out_rsq = nc.dram_tensor("rsq", [P, 2 * NCHUNK], f32, kind="ExternalOutput")

    with tile.TileContext(nc) as tc:
        with (
            tc.tile_pool(name="chunks", bufs=12) as chunk_pool,
            tc.tile_pool(name="sq", bufs=2) as sq_pool,
            tc.tile_pool(name="psum", bufs=1, space="PSUM") as psum_pool,
            tc.tile_pool(name="small", bufs=1) as small_pool,
        ):
            rsq_all = small_pool.tile([P, 2 * NCHUNK], f32)
            ones = small_pool.tile([P, 1], f32)
            nc.gpsimd.memset(ones, 1.0)
            ones_r = ones.bitcast(f32r)

            for m, src in enumerate(srcs):
                psum_mu = psum_pool.tile([1, D_PE], f32, tag=f"psum{m}")
                acc_hi = small_pool.tile([P, D - D_PE], f32, tag=f"acchi{m}")
                nc.gpsimd.memset(acc_hi, 0.0)
                for c in range(NCHUNK):
                    chunk = chunk_pool.tile([P, K_TILES * D], f32r)
                    nc.sync.dma_start(
                        out=chunk,
                        in_=src[c * CHUNK_ROWS : (c + 1) * CHUNK_ROWS, :]
                        .rearrange("(p k) d -> p (k d)", p=P)
                        .bitcast(f32r),
                    )
                    sq = sq_pool.tile([P, K_TILES * D], mybir.dt.bfloat16)
                    nc.scalar.activation(
                        out=sq,
                        in_=chunk.bitcast(f32),
                        func=mybir.ActivationFunctionType.Square,
                        accum_out=rsq_all[:, m * NCHUNK + c : m * NCHUNK + c + 1],
                    )
                    for k in range(K_TILES):
                        for j in range(D_PE // MM_N):
                            nc.tensor.matmul(
                                psum_mu[0:1, j * MM_N : (j + 1) * MM_N],
                                lhsT=ones_r,
                                rhs=chunk[:, k * D + j * MM_N : k * D + (j + 1) * MM_N],
                                start=(c == 0 and k == 0),
                                stop=(c == NCHUNK - 1 and k == K_TILES - 1),
                            )
                        nc.vector.tensor_add(
                            acc_hi,
                            acc_hi,
                            chunk[:, k * D + D_PE : (k + 1) * D].bitcast(f32),
                        )
                mu_sb = small_pool.tile([1, D_PE], f32, tag=f"mu{m}")
                nc.vector.tensor_copy(mu_sb, psum_mu)
                # Output DMAs go on the (idle) GpSimd SWDGE queue, emitted at
                # the end: the SP sequencer runs in order, so an output DMA
                # waiting mid-stream would stall the remaining input loads.
                nc.gpsimd.dma_start(out=out_mulo[m : m + 1, :], in_=mu_sb)
                nc.gpsimd.dma_start(out=out_acchi[m], in_=acc_hi)
            nc.gpsimd.dma_start(out=out_rsq[:, :], in_=rsq_all)
    nc.compile()
    return nc


def get_module():
    if "nc" not in _NC_CACHE:
        _NC_CACHE["nc"] = build_module()
    return _NC_CACHE["nc"]


def kernel(nfeats, afeats):
    nfeats = np.asarray(nfeats, dtype=np.float32)
    afeats = np.asarray(afeats, dtype=np.float32)
    assert nfeats.shape == (N_ROWS_FULL, D) and afeats.shape == (N_ROWS_FULL, D)

    nc = get_module()
    in_maps = [
        {
            "nfeats": np.ascontiguousarray(nfeats[c * ROWS : (c + 1) * ROWS]),
            "afeats": np.ascontiguousarray(afeats[c * ROWS : (c + 1) * ROWS]),
        }
        for c in range(N_CORES)
    ]
    results = run_bass_kernel_spmd(nc, in_maps, core_ids=list(range(N_CORES))).results

    mu = np.zeros((2, D), dtype=np.float64)
    sq = np.zeros(2, dtype=np.float64)
    for r in results:
        mu[:, :D_PE] += np.asarray(r["mulo"], dtype=np.float64)
        mu[:, D_PE:] += np.asarray(r["acchi"], dtype=np.float64).sum(axis=1)
        rsq = np.asarray(r["rsq"], dtype=np.float64)
        sq[0] += rsq[:, :NCHUNK].sum()
        sq[1] += rsq[:, NCHUNK:].sum()

    return combine(mu[0], mu[1], sq[0], sq[1])


def combine(mu_n, mu_a, sq_n, sq_a):
    nnum = anum = float(N_ROWS_FULL)
    nsum = nnum * sq_n - float(mu_n @ mu_n)
    asum = anum * sq_a - float(mu_a @ mu_a)
    cross_sum = anum * sq_n + nnum * sq_a - 2.0 * float(mu_n @ mu_a)

    ncount = nnum * (nnum - 1) / 2
    acount = anum * (anum - 1) / 2
    count = nnum * anum

    loss_dif = cross_sum / count
    within = (asum + nsum) / (acount + ncount)
    loss = -np.log(loss_dif / (loss_dif + within))
    return np.asarray(loss, dtype=np.float32)


# revision 15
# speedup vs baseline: 1.0671x; 1.0271x over previous
"""Trainium2 Bass kernel for nn_L2LossDif (pairwise L2 contrastive loss).

Math (see the algebraic reduction in the problem's reference):
    sq_m  = sum(feats_m ** 2)           (scalar, per matrix)
    mu_m  = feats_m.sum(axis=0)         ([D], per matrix)
then a handful of scalar ops combine sq_n, sq_a, mu_n, mu_a into the loss.

Strategy: data-parallel row shard across 8 cores (1024 rows of each matrix
per core). Each core streams its 16 MiB of rows once from HBM (2 MiB HWDGE
chunks). Per-chunk work is split so every engine runs faster than the DMA:
  - sum of squares     : ScalarE Square activation with accum_out
  - column sums 0:1024 : TensorE ones-matmul (float32r, 1 cyc/row) -> PSUM
  - column sums 1024:  : VectorE adds into a [128, 1024] accumulator
The DMA stream is the roofline. Partition/core reductions and the scalar
combine run on the host in float64.
"""

import numpy as np

import concourse.bacc as bacc
import concourse.mybir as mybir
import concourse.tile as tile
from concourse.bass_utils import run_bass_kernel_spmd

N_CORES = 8
N_ROWS_FULL = 8192
D = 2048
P = 128
ROWS = N_ROWS_FULL // N_CORES  # rows per core per matrix
K_TILES = 1  # 128-row tiles per DMA chunk (1 -> 1 MiB chunks)
CHUNK_ROWS = P * K_TILES
NCHUNK = ROWS // CHUNK_ROWS  # chunks per matrix
MM_N = 512  # moving free dim per matmul
D_PE = 1024  # columns summed on TensorE; the rest go to VectorE

_NC_CACHE = {}


def build_module():
    nc = bacc.Bacc("TRN2", target_bir_lowering=False, debug=False)
    f32 = mybir.dt.float32
    f32r = mybir.dt.float32r
    srcs = [
        nc.dram_tensor("nfeats", [ROWS, D], f32, kind="ExternalInput"),
        nc.dram_tensor("afeats", [ROWS, D], f32, kind="ExternalInput"),
    ]
    out_mulo = nc.dram_tensor("mulo", [2, D_PE], f32, kind="ExternalOutput")
    out_acchi = nc.dram_tensor("acchi", [2, P, D - D_PE], f32, kind="ExternalOutput")
    out_rsq = nc.dram_tensor("rsq", [P, 2 * NCHUNK], f32, kind="ExternalOutput")

    with tile.TileContext(nc) as tc:
        with (
            tc.tile_pool(name="chunks", bufs=8) as chunk_pool,
            tc.tile_pool(name="sq", bufs=2) as sq_pool,
            tc.tile_pool(name="psum", bufs=1, space="PSUM") as psum_pool,
            tc.tile_pool(name="small", bufs=1) as small_pool,
        ):
            rsq_all = small_pool.tile([P, 2 * NCHUNK], f32)
            ones = small_pool.tile([P, 1], f32)
            nc.gpsimd.memset(ones, 1.0)
            ones_r = ones.bitcast(f32r)

            for m, src in enumerate(srcs):
                psum_mu = psum_pool.tile([1, D_PE], f32, tag=f"psum{m}")
                acc_hi = small_pool.tile([P, D - D_PE], f32, tag=f"acchi{m}")
                nc.gpsimd.memset(acc_hi, 0.0)
                for c in range(NCHUNK):
                    chunk = chunk_pool.tile([P, K_TILES * D], f32r)
                    nc.sync.dma_start(
                        out=chunk,
                        in_=src[c * CHUNK_ROWS : (c + 1) * CHUNK_ROWS, :]
                        .rearrange("(p k) d -> p (k d)", p=P)
                        .bitcast(f32r),
                    )
                    sq = sq_pool.tile([P, K_TILES * D], mybir.dt.bfloat16)
                    nc.scalar.activation(
                        out=sq,
                        in_=chunk.bitcast(f32),
                        func=mybir.ActivationFunctionType.Square,
                        accum_out=rsq_all[:, m * NCHUNK + c : m * NCHUNK + c + 1],
                    )
                    for k in range(K_TILES):
                        for j in range(D_PE // MM_N):
                            nc.tensor.matmul(
                                psum_mu[0:1, j * MM_N : (j + 1) * MM_N],
                                lhsT=ones_r,
                                rhs=chunk[:, k * D + j * MM_N : k * D + (j + 1) * MM_N],
                                start=(c == 0 and k == 0),
                                stop=(c == NCHUNK - 1 and k == K_TILES - 1),
                            )
                        nc.vector.tensor_add(
                            acc_hi,
                            acc_hi,
                            chunk[:, k * D + D_PE : (k + 1) * D].bitcast(f32),
                        )
                mu_sb = small_pool.tile([1, D_PE], f32, tag=f"mu{m}")
                nc.vector.tensor_copy(mu_sb, psum_mu)
                # Output DMAs go on the (idle) GpSimd SWDGE queue, emitted at
                # the end: the SP sequencer runs in order, so an output DMA
                # waiting mid-stream would stall the remaining input loads.
                nc.gpsimd.dma_start(out=out_mulo[m : m + 1, :], in_=mu_sb)
                nc.gpsimd.dma_start(out=out_acchi[m], in_=acc_hi)
            nc.gpsimd.dma_start(out=out_rsq[:, :], in_=rsq_all)
    nc.compile()
    return nc


def get_module():
    if "nc" not in _NC_CACHE:
        _NC_CACHE["nc"] = build_module()
    return _NC_CACHE["nc"]


def kernel(nfeats, afeats):
    nfeats = np.asarray(nfeats, dtype=np.float32)
    afeats = np.asarray(afeats, dtype=np.float32)
    assert nfeats.shape == (N_ROWS_FULL, D) and afeats.shape == (N_ROWS_FULL, D)

    nc = get_module()
    in_maps = [
        {
            "nfeats": np.ascontiguousarray(nfeats[c * ROWS : (c + 1) * ROWS]),
            "afeats": np.ascontiguousarray(afeats[c * ROWS : (c + 1) * ROWS]),
        }
        for c in range(N_CORES)
    ]
    results = run_bass_kernel_spmd(nc, in_maps, core_ids=list(range(N_CORES))).results

    mu = np.zeros((2, D), dtype=np.float64)
    sq = np.zeros(2, dtype=np.float64)
    for r in results:
        mu[:, :D_PE] += np.asarray(r["mulo"], dtype=np.float64)
        mu[:, D_PE:] += np.asarray(r["acchi"], dtype=np.float64).sum(axis=1)
        rsq = np.asarray(r["rsq"], dtype=np.float64)
        sq[0] += rsq[:, :NCHUNK].sum()
        sq[1] += rsq[:, NCHUNK:].sum()

    return combine(mu[0], mu[1], sq[0], sq[1])


def combine(mu_n, mu_a, sq_n, sq_a):
    nnum = anum = float(N_ROWS_FULL)
    nsum = nnum * sq_n - float(mu_n @ mu_n)
    asum = anum * sq_a - float(mu_a @ mu_a)
    cross_sum = anum * sq_n + nnum * sq_a - 2.0 * float(mu_n @ mu_a)

    ncount = nnum * (nnum - 1) / 2
    acount = anum * (anum - 1) / 2
    count = nnum * anum

    loss_dif = cross_sum / count
    within = (asum + nsum) / (acount + ncount)
    loss = -np.log(loss_dif / (loss_dif + within))
    return np.asarray(loss, dtype=np.float32)
